# revision 5
# baseline (speedup 1.0000x reference)
"""v7: restructured epilogues + split-plane layout + offset-1536 BFP quant.

Same PE structure as v6b (64x64 quad tiles, 27-tap accumulation, fp16) but:

- Quantization via fp16-convert rounding at +1536/+1663 offsets:
    conv1: v = Relu(s1*ps + b1*s1) [ACT]; q = min(v,127)+1536 -> fp16 [DVE]
           (fp16 rounds to integer grid on [1024,2048))
    conv2: A = s2*ps + badj [ACT]; A~ = clamp(A,1536,1790) -> fp16 [DVE]
           z = A~ + xq [DVE TT]   (xq = host-quantized clip(round(s2*x)) - 1663)
           out = max(z,0)*inv_s2 -> fp16 [DVE]
  The +1536 activation offset flows through conv2's matmul; it is removed by a
  per-channel bias correction badj = b2*s2 + 1663 - s2*1536*sum(w2f) computed on
  host against the fp16-rounded weights. Plane borders are memset to 1536
  (= value 0 in offset space) so the correction is position-independent.

- Split-plane SBUF layout [128, 32, 58]: partition half0 stores padded plane
  rows [0..15]+[28..43], half1 rows [14..29]+[42..57]. Each PE quadrant reads
  moving data from its own half; spatial tiles are assigned so 6 (not 16)
  interior writes + 6 single-row halo writes build each quantized plane.
  x input planes use the same layout packed on host (halves the x DMA bytes).

- conv taps ordered kd-major with groups interleaved: conv2's kd=2 taps (which
  need the plane conv1 just produced) sit behind ~6us of kd=0/1 PE work.

- Output staged per timestep into [128, 4*392] SBUF and DMA'd contiguously.
"""

import numpy as np
import concourse.mybir as mybir
from concourse import bacc
from concourse.tile import TileContext
from concourse.bass_utils import run_bass_kernel_spmd

F16 = mybir.dt.float16
F32 = mybir.dt.float32

N, C, T, H, W = 8, 64, 16, 56, 56
TP = T + 2
SH, SW = 32, 58        # split-plane storage rows / cols
PLANE = SH * SW        # 1856 per half
ROWS = 7
NT = ROWS * W          # 392
NTAP = 27

# tile j on (group g, quadrant q): j = 4g + SIG[q]; SIG maps quadrant->tile idx
TPOS = [(0, 0), (0, 64), (64, 0), (64, 64)]
SIG = [0, 1, 3, 2]
# storage-row start (within half, before +16g) per quadrant
BS = [0, 7, 7, 0]
# psA holds q0 (part 0:64) + q3 (64:128); psB holds q2 (0:64) + q1 (64:128)
# block b = 2g + s (s=0:A, 1:B); tiles: p<64 -> TA[b], p>=64 -> TB[b]
TA = [0, 3, 4, 7]
TB = [2, 1, 6, 5]

# halo single-row writes per conv1 output plane:
# (g, src_ps 'A'/'B', src_part_half, src_row, dst_half, dst_strow)
HALO = [
    (0, 'A', 1, 0, 0, 15),   # plane row 15 = tile2 row0 -> half0 st15
    (0, 'B', 1, 6, 1, 0),    # plane row 14 = tile1 row6 -> half1 st0
    (0, 'B', 0, 6, 0, 16),   # plane row 28 = tile3 row6 -> half0 st16
    (1, 'A', 0, 0, 1, 15),   # plane row 29 = tile4 row0 -> half1 st15
    (1, 'A', 1, 0, 0, 31),   # plane row 43 = tile6 row0 -> half0 st31
    (1, 'B', 1, 6, 1, 16),   # plane row 42 = tile5 row6 -> half1 st16
]

TAPS_STD = [(kh, kw) for kh in range(3) for kw in range(3)]
TAPS_KW1_LAST = ([(kh, kw) for kh in range(3) for kw in (0, 2)]
                 + [(kh, 1) for kh in range(3)])

_COMPILED = None


def _build():
    nc = bacc.Bacc()
    xpad_d = nc.declare_dram_parameter("xpad", [128, TP, PLANE], F16, isOutput=False)
    # shifted x planes (cols 1..56 at row pitch 56) for 4B-aligned kw=1 taps
    xpads_d = nc.declare_dram_parameter("xpads", [128, TP, SH * W], F16,
                                        isOutput=False)
    xq_d = nc.declare_dram_parameter("xq", [128, T, 4 * NT], F16, isOutput=False)
    # w1 split by kd so the first matmuls only wait on a 1/3 slice
    w1_d = [
        nc.declare_dram_parameter(f"w1kd{k}", [128, 9 * 64], F16, isOutput=False)
        for k in range(3)
    ]
    w2_d = nc.declare_dram_parameter("w2p", [128, NTAP * 64], F16, isOutput=False)
    coeff_d = nc.declare_dram_parameter("coeff", [128, 7], F32, isOutput=False)
    out_d = nc.declare_dram_parameter("out", [128, T * 4 * NT], F16, isOutput=True)

    def sview(ap):
        return ap.rearrange("p (r c) -> p r c", c=SW)

    with TileContext(nc) as tc:
        with (
            tc.tile_pool(name="big", bufs=1) as bigpool,
            tc.tile_pool(name="xd", bufs=5) as xpool,
            tc.tile_pool(name="xs", bufs=5) as xspool,
            tc.tile_pool(name="qd", bufs=4) as qpool,
            tc.tile_pool(name="qs", bufs=4) as qspool,
            tc.tile_pool(name="xq", bufs=3) as xqpool,
            tc.tile_pool(name="v", bufs=4) as vpool,
            tc.tile_pool(name="a", bufs=4) as apool,
            tc.tile_pool(name="at", bufs=4) as atpool,
            tc.tile_pool(name="z", bufs=4) as zpool,
            tc.tile_pool(name="os", bufs=2) as opool,
            tc.tile_pool(name="ps1", bufs=4, space="PSUM") as ps1pool,
            tc.tile_pool(name="ps2", bufs=4, space="PSUM") as ps2pool,
        ):
            xpl = {}
            xpls = {}

            def load_xp(s):
                xt_ = xpool.tile([128, PLANE], F16, tag="xpl")
                nc.sync.dma_start(out=xt_[:], in_=xpad_d[:, s, :])
                xpl[s] = xt_

            def load_xs(s):
                xs_ = xspool.tile([128, SH * W], F16, tag="xpls")
                nc.sync.dma_start(out=xs_[:], in_=xpads_d[:, s, :])
                xpls[s] = xs_

            def load_x(s):
                load_xp(s)
                load_xs(s)

            # HAM warmup fodder: no-DMA-dependency dummy matmuls that run
            # while the startup DMAs land, so real matmuls start at 2.4GHz.
            scr = bigpool.tile([128, NT], F16, tag="scr")
            nc.gpsimd.memset(scr[:], 0.0)

            # conv1(0) skips kd=0 (x plane 0 is the zero pad), so plane 1
            # and w1 kd-slices 1,2 are the startup critical path; plane 1's
            # shifted copy is covered by t=0 running its kw!=1 taps first.
            load_xp(1)
            w1 = [None, None, None]
            w1k1 = bigpool.tile([128, 9 * 64], F16, tag="w1k1")
            nc.sync.dma_start(out=w1k1[:], in_=w1_d[1][:])
            w1[1] = w1k1
            load_xs(1)
            w1k2 = bigpool.tile([128, 9 * 64], F16, tag="w1k2")
            nc.sync.dma_start(out=w1k2[:], in_=w1_d[2][:])
            w1[2] = w1k2
            coeff = bigpool.tile([128, 7], F32, tag="coeff")
            nc.sync.dma_start(out=coeff[:], in_=coeff_d[:])
            load_x(2)
            w1k0 = bigpool.tile([128, 9 * 64], F16, tag="w1k0")
            nc.sync.dma_start(out=w1k0[:], in_=w1_d[0][:])
            w1[0] = w1k0

            ps_warm = ps2pool.tile([128, NT], F32, tag="ps2")
            for _ in range(12):
                nc.tensor.matmul(
                    ps_warm[0:64, :], scr[0:64, 0:64], scr[0:64, :],
                    start=True, stop=True,
                    tile_position=(0, 0), skip_group_check=True,
                )

            s1 = coeff[:, 0:1]
            b1s1 = coeff[:, 1:2]
            s2 = coeff[:, 2:3]
            badj27 = coeff[:, 3:4]
            badj18 = coeff[:, 4:5]
            inv_s2 = coeff[:, 5:6]
            badj0 = coeff[:, 6:7]

            # qdup[k] = quantized conv1 output plane k-1 (offset +1536);
            # plane 0 (temporal zero pad) is skipped via badj0 instead.
            # qsh[k] = same plane shifted left one column (cols 1..56 of
            # qdup stored at 0..55, row pitch 56) so conv2's kw=1 taps
            # stream 4B-aligned rows instead of paying the odd-element
            # fetch penalty (~6ns/group measured).
            qdup = {}
            qsh = {}

            def plane_border(qp):
                v = sview(qp[:])
                nc.gpsimd.memset(v[0:64, 0, :], 1536.0)
                nc.gpsimd.memset(v[64:128, SH - 1, :], 1536.0)
                nc.gpsimd.memset(v[:, :, 0], 1536.0)
                nc.gpsimd.memset(v[:, :, SW - 1], 1536.0)

            def conv_mms(items, ps):
                # kd-major, groups interleaved; 4 quadrants per (tap, g)
                # items: (w_tile, col_base, plane_view) per kd, emission order
                nk = len(items)
                for e, (wt, cb, pv, pvs, taps) in enumerate(items):
                    for g in range(2):
                        psA, psB = ps[g]
                        for ti, (kh, kw) in enumerate(taps):
                            if True:
                                j0 = kh * 3 + kw
                                j = cb + j0
                                for q in range(4):
                                    hf = q // 2
                                    wsl = wt[64 * hf: 64 * hf + 64,
                                             64 * j: 64 * j + 64]
                                    if kw == 1 and pvs is not None:
                                        mv = pvs[64 * hf: 64 * hf + 64,
                                                 16 * g + BS[q] + kh:
                                                 16 * g + BS[q] + kh + ROWS,
                                                 0: W]
                                    else:
                                        mv = pv[64 * hf: 64 * hf + 64,
                                                16 * g + BS[q] + kh:
                                                16 * g + BS[q] + kh + ROWS,
                                                kw: kw + W]
                                    pst = psA if q in (0, 3) else psB
                                    out_ap = (pst[0:64, :] if q in (0, 2)
                                              else pst[64:128, :])
                                    nc.tensor.matmul(
                                        out_ap, wsl, mv,
                                        start=(e == 0 and ti == 0),
                                        stop=(e == nk - 1 and ti == 8),
                                        tile_position=TPOS[q],
                                        skip_group_check=True,
                                    )

            w2 = None
            for t in range(T + 1):
                if t < T:
                    if t + 3 <= TP - 2:
                        load_x(t + 3)
                    qp = qpool.tile([128, PLANE], F16, tag="qdup")
                    plane_border(qp)
                    qdup[t + 1] = qp
                    qv = sview(qp[:])
                    qs = qspool.tile([128, SH * W], F16, tag="qsh")
                    qsh[t + 1] = qs
                    qsv = qs[:].rearrange("p (r c) -> p r c", c=W)
                    nc.gpsimd.memset(qsv[0:64, 0, :], 1536.0)
                    nc.gpsimd.memset(qsv[64:128, SH - 1, :], 1536.0)

                    # temporal zero-pad planes contribute nothing: skip them
                    kds1 = [1, 2] if t == 0 else ([0, 1] if t == T - 1
                                                  else [0, 1, 2])
                    # t=0: kw!=1 taps first to cover plane 1's shifted-copy DMA
                    taps1 = (TAPS_KW1_LAST if t == 0 else TAPS_STD)
                    ps1 = []
                    for g in range(2):
                        psA = ps1pool.tile([128, NT], F32, tag="ps1")
                        psB = ps1pool.tile([128, NT], F32, tag="ps1")
                        ps1.append((psA, psB))
                    conv_mms([(w1[kd], 0, sview(xpl[t + kd][:]),
                               xpls[t + kd][:].rearrange(
                                   "p (r c) -> p r c", c=W), taps1)
                              for kd in kds1], ps1)
                    if t == 0:
                        # w2 deferred off the startup critical path
                        w2 = bigpool.tile([128, NTAP * 64], F16, tag="w2")
                        nc.sync.dma_start(out=w2[:], in_=w2_d[:])

                    vs = {}
                    for g in range(2):
                        psA, psB = ps1[g]
                        vA = vpool.tile([128, NT], F32, tag="v")
                        nc.scalar.activation(
                            vA[:], psA[:], mybir.ActivationFunctionType.Relu,
                            bias=b1s1, scale=s1,
                        )
                        vB = vpool.tile([128, NT], F32, tag="v")
                        nc.scalar.activation(
                            vB[:], psB[:], mybir.ActivationFunctionType.Relu,
                            bias=b1s1, scale=s1,
                        )
                        vs[(g, 'A')] = vA
                        vs[(g, 'B')] = vB
                        vAv = vA[:].rearrange("p (r w) -> p r w", w=W)
                        vBv = vB[:].rearrange("p (r w) -> p r w", w=W)
                        # psA: both halves land at same storage rows
                        nc.vector.tensor_scalar(
                            out=qv[:, 16 * g + 1: 16 * g + 8, 1: 1 + W],
                            in0=vAv, scalar1=127.0, scalar2=1536.0,
                            op0=mybir.AluOpType.min, op1=mybir.AluOpType.add,
                        )
                        nc.vector.tensor_scalar(
                            out=qsv[:, 16 * g + 1: 16 * g + 8, 0: W],
                            in0=vAv, scalar1=127.0, scalar2=1536.0,
                            op0=mybir.AluOpType.min, op1=mybir.AluOpType.add,
                        )
                        # psB: crossed halves
                        nc.vector.tensor_scalar(
                            out=qv[64:128, 16 * g + 8: 16 * g + 15, 1: 1 + W],
                            in0=vBv[0:64], scalar1=127.0, scalar2=1536.0,
                            op0=mybir.AluOpType.min, op1=mybir.AluOpType.add,
                        )
                        nc.vector.tensor_scalar(
                            out=qsv[64:128, 16 * g + 8: 16 * g + 15, 0: W],
                            in0=vBv[0:64], scalar1=127.0, scalar2=1536.0,
                            op0=mybir.AluOpType.min, op1=mybir.AluOpType.add,
                        )
                        nc.vector.tensor_scalar(
                            out=qv[0:64, 16 * g + 8: 16 * g + 15, 1: 1 + W],
                            in0=vBv[64:128], scalar1=127.0, scalar2=1536.0,
                            op0=mybir.AluOpType.min, op1=mybir.AluOpType.add,
                        )
                        nc.vector.tensor_scalar(
                            out=qsv[0:64, 16 * g + 8: 16 * g + 15, 0: W],
                            in0=vBv[64:128], scalar1=127.0, scalar2=1536.0,
                            op0=mybir.AluOpType.min, op1=mybir.AluOpType.add,
                        )
                    for (g, sp, sh, srow, dh, drow) in HALO:
                        sv = vs[(g, sp)][:].rearrange("p (r w) -> p r w", w=W)
                        nc.vector.tensor_scalar(
                            out=qv[64 * dh: 64 * dh + 64, drow: drow + 1, 1: 1 + W],
                            in0=sv[64 * sh: 64 * sh + 64, srow: srow + 1, :],
                            scalar1=127.0, scalar2=1536.0,
                            op0=mybir.AluOpType.min, op1=mybir.AluOpType.add,
                        )
                        nc.vector.tensor_scalar(
                            out=qsv[64 * dh: 64 * dh + 64, drow: drow + 1, 0: W],
                            in0=sv[64 * sh: 64 * sh + 64, srow: srow + 1, :],
                            scalar1=127.0, scalar2=1536.0,
                            op0=mybir.AluOpType.min, op1=mybir.AluOpType.add,
                        )

                    # prefetch xq for conv2(t) used next iteration
                    xqt = xqpool.tile([128, 4 * NT], F16, tag="xq")
                    nc.sync.dma_start(out=xqt[:], in_=xq_d[:, t, :])
                    if t == 0:
                        xq_tiles = {}
                    xq_tiles[t] = xqt

                if t >= 1:
                    u = t - 1
                    # u=0: kd0 plane is the all-1536 pad -> fold into badj0;
                    # u=15: kd2 plane absent (zero pad) -> badj18
                    kds2 = [1, 2] if u == 0 else ([0, 1] if u == T - 1
                                                  else [0, 1, 2])
                    ps2 = []
                    for g in range(2):
                        psA = ps2pool.tile([128, NT], F32, tag="ps2")
                        psB = ps2pool.tile([128, NT], F32, tag="ps2")
                        ps2.append((psA, psB))
                    conv_mms([(w2, 9 * kd, sview(qdup[u + kd][:]),
                               qsh[u + kd][:].rearrange("p (r c) -> p r c",
                                                        c=W), TAPS_STD)
                              for kd in kds2], ps2)

                    badj = (badj0 if u == 0
                            else badj18 if u == T - 1 else badj27)
                    xqt = xq_tiles.pop(u)
                    ostage = opool.tile([128, 4 * NT], F16, tag="os")
                    for g in range(2):
                        for s, pst in enumerate(ps2[g]):
                            b = 2 * g + s
                            A = apool.tile([128, NT], F32, tag="A")
                            nc.scalar.activation(
                                A[:], pst[:],
                                mybir.ActivationFunctionType.Identity,
                                bias=badj, scale=s2,
                            )
                            At = atpool.tile([128, NT], F16, tag="At")
                            nc.vector.tensor_scalar(
                                out=At[:], in0=A[:],
                                scalar1=1536.0, scalar2=1790.0,
                                op0=mybir.AluOpType.max, op1=mybir.AluOpType.min,
                            )
                            z = zpool.tile([128, NT], F16, tag="z")
                            nc.vector.tensor_tensor(
                                out=z[:], in0=At[:],
                                in1=xqt[:, b * NT: (b + 1) * NT],
                                op=mybir.AluOpType.add,
                            )
                            nc.vector.tensor_scalar(
                                out=ostage[:, b * NT: (b + 1) * NT], in0=z[:],
                                scalar1=0.0, scalar2=inv_s2,
                                op0=mybir.AluOpType.max, op1=mybir.AluOpType.mult,
                            )
                            if u == T - 1:
                                # last timestep: drain per block
                                ob = u * 4 * NT + b * NT
                                nc.sync.dma_start(
                                    out=out_d[:, ob: ob + NT],
                                    in_=ostage[:, b * NT: (b + 1) * NT],
                                )
                        if u < T - 1:
                            # per-group half DMA so the tail drains early
                            o0 = u * 4 * NT + g * 2 * NT
                            nc.sync.dma_start(
                                out=out_d[:, o0: o0 + 2 * NT],
                                in_=ostage[:, g * 2 * NT: (g + 1) * 2 * NT],
                            )
    nc.compile()
    return nc


def _host_pack(x, w1, b1, w2, b2, exp1, exp2):
    scale1 = np.exp2(7.0 - exp1.astype(np.float64))
    scale2 = np.exp2(7.0 - exp2.astype(np.float64))

    def pack_w(wt):
        # wt: [kd,kh,kw,i,o] -> [128, 27*64] fp16 (dup halves)
        p = wt.reshape(NTAP, 64, 64)
        p = np.ascontiguousarray(np.transpose(p, (1, 0, 2))).reshape(64, NTAP * 64)
        p16 = p.astype(np.float16)
        return np.concatenate([p16, p16], axis=0)

    w1t = np.transpose(w1, (2, 3, 4, 1, 0)).astype(np.float32)
    w1p = pack_w(w1t)
    w1kd = w1p.reshape(128, 3, 9 * 64)
    w2f = (w2.astype(np.float64) / scale1[None, :, None, None, None])
    w2t = np.transpose(w2f, (2, 3, 4, 1, 0)).astype(np.float32)
    w2p = pack_w(w2t)

    # offset corrections against fp16-rounded w2f: [kd,kh,kw,i,o]
    w2t16 = w2t.astype(np.float16).astype(np.float64).reshape(3, 9 * 64, 64)
    off27 = 1536.0 * w2t16.sum(axis=(0, 1))
    off18 = 1536.0 * w2t16[:2].sum(axis=(0, 1))
    off12 = 1536.0 * w2t16[1:].sum(axis=(0, 1))

    c64 = np.zeros((64, 7), dtype=np.float64)
    c64[:, 0] = scale1
    c64[:, 1] = b1.astype(np.float64) * scale1
    c64[:, 2] = scale2
    c64[:, 3] = b2.astype(np.float64) * scale2 + 1663.0 - scale2 * off27
    c64[:, 4] = b2.astype(np.float64) * scale2 + 1663.0 - scale2 * off18
    c64[:, 5] = 1.0 / scale2
    c64[:, 6] = b2.astype(np.float64) * scale2 + 1663.0 - scale2 * off12
    coeff = np.concatenate([c64, c64], axis=0).astype(np.float32)

    idx0 = list(range(0, 16)) + list(range(28, 44))
    idx1 = list(range(14, 30)) + list(range(42, 58))

    shared = {"w2p": w2p, "coeff": coeff}
    for k in range(3):
        shared[f"w1kd{k}"] = np.ascontiguousarray(w1kd[:, k, :])
    in_maps = []
    for n in range(N):
        xp = np.pad(x[n], ((0, 0), (1, 1), (1, 1), (1, 1))).astype(np.float16)
        xs = np.stack([xp[:, :, idx0, :], xp[:, :, idx1, :]], axis=0)
        m = dict(shared)
        m["xpad"] = np.ascontiguousarray(xs.reshape(128, TP, PLANE))
        m["xpads"] = np.ascontiguousarray(
            xs[:, :, :, :, 1:57].reshape(128, TP, SH * W))

        cX = np.clip(np.round(x[n].astype(np.float64)
                              * scale2[:, None, None, None]), -127, 127)
        xq = (cX - 1663.0).astype(np.float16).reshape(C, T, 8, ROWS, W)
        xqp = np.stack([xq[:, :, TA], xq[:, :, TB]], axis=0)
        m["xq"] = np.ascontiguousarray(xqp.reshape(128, T, 4 * NT))
        in_maps.append(m)
    return in_maps


def kernel(x, w1, b1, w2, b2, exp1, exp2):
    global _COMPILED
    x = np.asarray(x, dtype=np.float32)
    w1 = np.asarray(w1, dtype=np.float32)
    b1 = np.asarray(b1, dtype=np.float32)
    w2 = np.asarray(w2, dtype=np.float32)
    b2 = np.asarray(b2, dtype=np.float32)
    exp1 = np.asarray(exp1, dtype=np.float32)
    exp2 = np.asarray(exp2, dtype=np.float32)
    if _COMPILED is None:
        _COMPILED = _build()
    in_maps = _host_pack(x, w1, b1, w2, b2, exp1, exp2)
    res = run_bass_kernel_spmd(_COMPILED, in_maps, core_ids=list(range(N)))
    out = np.empty((N, C, T, H, W), dtype=np.float32)
    for n in range(N):
        od = np.asarray(res.results[n]["out"], dtype=np.float32)
        od = od.reshape(2, 64, T, 4, ROWS, W)
        full = np.empty((C, T, 8, ROWS, W), dtype=np.float32)
        for half, tbl in ((0, TA), (1, TB)):
            for b in range(4):
                full[:, :, tbl[b]] = od[half, :, :, b]
        out[n] = full.reshape(C, T, H, W)
    return out
